# revision 17
# baseline (speedup 1.0000x reference)
"""Trainium2 Bass kernel for AttentionGRUDecoder (Bahdanau attention + GRU cell + FC head).

B=256, S=2048, H=256, f32 inputs. Data-parallel over batch across 8 NeuronCores
(32 batches/core). Weights replicated; small weights pre-transposed on host.

Per-core dataflow (per batch b):
  enc[b] [2048,256] f32  --DMA-->  SBUF natural (s = p*16+t permutation, benign)
     --DVE cast--> fp16  --xbar DMA transpose--> encT [h,s] fp16
  kT = WkT.T @ encT (PE, fp16)  --ACT tanh(+q+attn_b bias)--> t [h',s] fp16
  score = WsT.T @ t (PE) -> PSUM rows [G,2048] (one row per batch)
  softmax rows (DVE max / ACT exp+accum / DVE recip+scale)
  context^T[:,b] = sum_s encT * attn  (DVE scalar_tensor_tensor accum_out)
  GRU + FC on [h,b]-transposed layout (PE + DVE + ACT), outputs [H,BL] + [BL,1]
"""

import os
import sys

import numpy as np

sys.path.insert(0, "/opt/trn_rl_repo")

import concourse.bass as bass
import concourse.bacc as bacc
import concourse.mybir as mybir
import concourse.tile as tile

F32 = mybir.dt.float32
F16 = mybir.dt.float16
AX = mybir.AxisListType
ALU = mybir.AluOpType
ACT = mybir.ActivationFunctionType

B, S, H = 256, 2048, 256
NCORES = 8
BL = B // NCORES  # 32 batches per core
G = 8             # batches per softmax/context group (SBUF residency)

# Set False to materialize attn rows via gpsimd if broadcast-view APs fail.
BCAST_VIEW = False


def build_nc(bl=BL, s=S, g=G):
    """Build the per-core Bass program (SPMD: same program, sharded inputs)."""
    nc = bacc.Bacc("TRN2", target_bir_lowering=False, debug=False)
    st = s // 128   # number of 128-row s-tiles (16)
    ngrp = bl // g

    # ---- DRAM I/O ----
    enc_d = nc.declare_dram_parameter("enc", [bl, s, H], F32, isOutput=False)
    ypT_d = nc.declare_dram_parameter("ypT", [H, bl], F32, isOutput=False)
    hidT_d = nc.declare_dram_parameter("hidT", [H, bl], F32, isOutput=False)
    wkT_d = nc.declare_dram_parameter("wkT", [H, H], F16, isOutput=False)
    wqT_d = nc.declare_dram_parameter("wqT", [H, H], F16, isOutput=False)
    attnb_d = nc.declare_dram_parameter("attnb", [2, 128], F32, isOutput=False)
    wsT_d = nc.declare_dram_parameter("wsT", [2, 128], F16, isOutput=False)
    wihT_d = nc.declare_dram_parameter("wihT", [H, 3 * H], F16, isOutput=False)
    whhT_d = nc.declare_dram_parameter("whhT", [H, 3 * H], F16, isOutput=False)
    bsum_d = nc.declare_dram_parameter("bsum", [6, 128], F32, isOutput=False)
    bihn_d = nc.declare_dram_parameter("bihn", [2, 128], F32, isOutput=False)
    bhhn_d = nc.declare_dram_parameter("bhhn", [2, 128], F32, isOutput=False)
    wfc_d = nc.declare_dram_parameter("wfc", [2, 128], F32, isOutput=False)
    bfc_d = nc.declare_dram_parameter("bfc", [bl, 1], F32, isOutput=False)
    ident_d = nc.declare_dram_parameter("ident", [128, 128], F16, isOutput=False)
    hnewT_d = nc.declare_dram_parameter("hnewT", [H, bl], F32, isOutput=True)
    fc_d = nc.declare_dram_parameter("fc", [bl, 1], F32, isOutput=True)

    with tile.TileContext(nc) as tc:
        with tc.tile_pool(name="const", bufs=1) as cp:
            # ---- constants into SBUF ----
            wkT = cp.tile([128, 2, H], F16)      # [k-part, kc, h'] lhsT for kT matmul
            nc.scalar.dma_start(wkT[:], wkT_d.rearrange("(kc p) m -> p kc m", p=128))
            wqT = cp.tile([128, 2, H], F16)
            nc.scalar.dma_start(wqT[:], wqT_d.rearrange("(kc p) m -> p kc m", p=128))
            attnb = cp.tile([128, 2], F32)
            nc.scalar.dma_start(attnb[:], attnb_d.rearrange("c p -> p c"))
            wsT = cp.tile([128, 2], F16)
            nc.scalar.dma_start(wsT[:], wsT_d.rearrange("c p -> p c"))
            wihT = cp.tile([128, 2, 3 * H], F16)
            nc.scalar.dma_start(wihT[:], wihT_d.rearrange("(kc p) m -> p kc m", p=128))
            whhT = cp.tile([128, 2, 3 * H], F16)
            nc.scalar.dma_start(whhT[:], whhT_d.rearrange("(kc p) m -> p kc m", p=128))
            bsum = cp.tile([128, 6], F32)
            nc.scalar.dma_start(bsum[:], bsum_d.rearrange("c p -> p c"))
            bihn = cp.tile([128, 2], F32)
            nc.scalar.dma_start(bihn[:], bihn_d.rearrange("c p -> p c"))
            bhhn = cp.tile([128, 2], F32)
            nc.scalar.dma_start(bhhn[:], bhhn_d.rearrange("c p -> p c"))
            wfc = cp.tile([128, 2], F32)
            nc.scalar.dma_start(wfc[:], wfc_d.rearrange("c p -> p c"))
            bfc = cp.tile([bl, 1], F32)
            nc.scalar.dma_start(bfc[:], bfc_d[:])
            ypT = cp.tile([128, 2, bl], F32)
            nc.scalar.dma_start(ypT[:], ypT_d.rearrange("(c p) b -> p c b", p=128))
            hidT = cp.tile([128, 2, bl], F32)
            nc.scalar.dma_start(hidT[:], hidT_d.rearrange("(c p) b -> p c b", p=128))

            ident = cp.tile([128, 128], F16)
            nc.scalar.dma_start(ident[:], ident_d[:])
            hidT16 = cp.tile([128, 2, bl], F16)
            nc.vector.tensor_copy(hidT16[:], hidT[:])

            # bias_qb[h', c, b] = q^T + attn_b  (tanh bias, per-partition cols)
            bias_qb = cp.tile([128, 2, bl], F32)
            with tc.tile_pool(name="qpsum", bufs=2, space="PSUM") as qp:
                for c in range(2):
                    q_ps = qp.tile([128, bl], F32)
                    for kc in range(2):
                        nc.tensor.matmul(
                            q_ps[:],
                            wqT[:, kc, c * 128:(c + 1) * 128],
                            hidT16[:, kc, :],
                            start=(kc == 0), stop=(kc == 1),
                        )
                    nc.scalar.activation(
                        bias_qb[:, c, :], q_ps[:], ACT.Identity,
                        bias=attnb[:, c:c + 1],
                    )

            # context^T accumulator: [h-part, c*bl+b]
            ctxT = cp.tile([128, 2 * bl], F32)

            # ---- attention over batches ----
            with (
                tc.tile_pool(name="encf32", bufs=2) as pf32,
                tc.tile_pool(name="encf16", bufs=2) as pf16,
                tc.tile_pool(name="encT", bufs=g) as pT,
                tc.tile_pool(name="tpool", bufs=2) as pt,
                tc.tile_pool(name="attn", bufs=1) as pa,
                tc.tile_pool(name="scr", bufs=1) as psc,
                tc.tile_pool(name="stg", bufs=2) as pstg,
                tc.tile_pool(name="abc", bufs=2) as pabc,
                tc.tile_pool(name="ktpsum", bufs=2, space="PSUM") as kt_pool,
                tc.tile_pool(name="scpsum", bufs=1, space="PSUM") as sc_pool,
                tc.tile_pool(name="tppsum", bufs=2, space="PSUM") as tp_pool,
                tc.tile_pool(name="scdram", bufs=1, space="DRAM") as sc_dram,
            ):
                scd = sc_dram.tile([bl, s], F32)
                attnd = sc_dram.tile([bl, s], F16)
                for grp in range(ngrp):
                    encT_list = []
                    for j in range(g):
                        b = grp * g + j
                        # load natural, casting f32->f16 in the SWDGE DMA
                        # (s = p*st + t permutation, benign)
                        ef16 = pf16.tile([128, st, H], F16, tag="ef16")
                        nc.gpsimd.dma_start(
                            ef16[:], enc_d[b].rearrange("(p t) h -> p t h", t=st))
                        # PE transpose (identity matmul) -> PSUM -> copyback
                        eT = pT.tile([128, 2, s], F16, tag="encT")
                        encT_list.append(eT)
                        for tq in range(st // 4):
                            for c in range(2):
                                tp_ps = tp_pool.tile([128, 512], F16, tag="tp")
                                for i in range(4):
                                    t = tq * 4 + i
                                    nc.tensor.transpose(
                                        tp_ps[:, i * 128:(i + 1) * 128],
                                        ef16[:, t, c * 128:(c + 1) * 128],
                                        ident[:],
                                    )
                                eng = nc.vector.tensor_copy if (tq + c) % 2 else nc.scalar.copy
                                eng(eT[:, c, tq * 512:(tq + 1) * 512], tp_ps[:])
                        # kT matmul + tanh -> t tile
                        tt = pt.tile([128, 2, s], F16, tag="tt")
                        for c in range(2):
                            for q in range(s // 1024):
                                kt = kt_pool.tile([128, 1024], F32, tag="kt")
                                for qq in range(2):
                                    sl = slice(q * 1024 + qq * 512,
                                               q * 1024 + (qq + 1) * 512)
                                    for kc in range(2):
                                        nc.tensor.matmul(
                                            kt[:, qq * 512:(qq + 1) * 512],
                                            wkT[:, kc, c * 128:(c + 1) * 128],
                                            eT[:, kc, sl],
                                            start=(kc == 0), stop=(kc == 1),
                                        )
                                nc.scalar.activation(
                                    tt[:, c, q * 1024:(q + 1) * 1024], kt[:],
                                    ACT.Tanh, bias=bias_qb[:, c, b:b + 1],
                                )
                        # score row: Ws^T @ t -> psum [1,1024] halves -> SBUF -> DRAM
                        stg = pstg.tile([1, s], F32, tag="stg")
                        for half in range(2):
                            sc_ps = sc_pool.tile([1, 1024], F32, tag="scps")
                            for qq in range(2):
                                sl_p = slice(qq * 512, (qq + 1) * 512)
                                sl_t = slice(half * 1024 + qq * 512,
                                             half * 1024 + (qq + 1) * 512)
                                for kc in range(2):
                                    nc.tensor.matmul(
                                        sc_ps[0:1, sl_p],
                                        wsT[:, kc:kc + 1],
                                        tt[:, kc, sl_t],
                                        start=(kc == 0), stop=(kc == 1),
                                    )
                            nc.scalar.activation(
                                stg[0:1, half * 1024:(half + 1) * 1024],
                                sc_ps[:], ACT.Copy)
                        nc.scalar.dma_start(scd[b:b + 1, :], stg[:])
                    # ---- group softmax ----
                    scg = pa.tile([g, s], F32, tag="scg")
                    nc.scalar.dma_start(scg[:], scd[grp * g:(grp + 1) * g, :])
                    mx = pa.tile([g, 1], F32, tag="mx")
                    nc.vector.reduce_max(mx[:], scg[:], axis=AX.X)
                    nmx = pa.tile([g, 1], F32, tag="nmx")
                    nc.vector.tensor_scalar_mul(nmx[:], mx[:], -1.0)
                    expt = pa.tile([g, s], F16, tag="expt")
                    sume = pa.tile([g, 1], F32, tag="sume")
                    nc.scalar.activation(expt[:], scg[:], ACT.Exp,
                                         bias=nmx[:], accum_out=sume[:])
                    inv = pa.tile([g, 1], F32, tag="inv")
                    nc.vector.reciprocal(inv[:], sume[:])
                    attn = pa.tile([g, s], F16, tag="attn")
                    nc.vector.tensor_scalar_mul(attn[:], expt[:], inv[:])
                    nc.scalar.dma_start(attnd[grp * g:(grp + 1) * g, :], attn[:])
                    # ---- context per batch ----
                    for j in range(g):
                        b = grp * g + j
                        for c in range(2):
                            scr = psc.tile([128, s], F16, tag="scr")
                            if c == 0:
                                a_bc = pabc.tile([128, s], F16, tag="abc")
                                nc.scalar.dma_start(
                                    a_bc[:], attnd[b:b + 1, :].broadcast_to([128, s]))
                            a_row = a_bc[:]
                            nc.vector.scalar_tensor_tensor(
                                scr[:], encT_list[j][:, c, :], 1.0, a_row,
                                op0=ALU.mult, op1=ALU.mult,
                                accum_out=ctxT[:, c * bl + b:c * bl + b + 1],
                            )

            # ---- GRU cell + FC head (transposed [h, b] layout) ----
            with (
                tc.tile_pool(name="gru", bufs=1) as gp,
                tc.tile_pool(name="grupsum", bufs=2, space="PSUM") as gps,
            ):
                xT16 = gp.tile([128, 2, bl], F16)
                for c in range(2):
                    nc.vector.tensor_add(xT16[:, c, :],
                                         ctxT[:, c * bl:(c + 1) * bl],
                                         ypT[:, c, :])
                rz = gp.tile([128, 4, bl], F32)
                gin = gp.tile([128, 2, bl], F32)
                ghn = gp.tile([128, 2, bl], F32)
                for c6 in range(6):
                    if c6 < 4:
                        # gi+gh accumulate in one PSUM group (4 matmuls)
                        g_ps = gps.tile([128, bl], F32, tag="gi")
                        for kc in range(2):
                            nc.tensor.matmul(g_ps[:],
                                             wihT[:, kc, c6 * 128:(c6 + 1) * 128],
                                             xT16[:, kc, :],
                                             start=(kc == 0), stop=False)
                        for kc in range(2):
                            nc.tensor.matmul(g_ps[:],
                                             whhT[:, kc, c6 * 128:(c6 + 1) * 128],
                                             hidT16[:, kc, :],
                                             start=False, stop=(kc == 1))
                        nc.scalar.activation(rz[:, c6, :], g_ps[:], ACT.Sigmoid,
                                             bias=bsum[:, c6:c6 + 1])
                    else:
                        gi_ps = gps.tile([128, bl], F32, tag="gi")
                        gh_ps = gps.tile([128, bl], F32, tag="gh")
                        for kc in range(2):
                            nc.tensor.matmul(gi_ps[:],
                                             wihT[:, kc, c6 * 128:(c6 + 1) * 128],
                                             xT16[:, kc, :],
                                             start=(kc == 0), stop=(kc == 1))
                        for kc in range(2):
                            nc.tensor.matmul(gh_ps[:],
                                             whhT[:, kc, c6 * 128:(c6 + 1) * 128],
                                             hidT16[:, kc, :],
                                             start=(kc == 0), stop=(kc == 1))
                        c = c6 - 4
                        nc.scalar.activation(gin[:, c, :], gi_ps[:], ACT.Identity,
                                             bias=bihn[:, c:c + 1])
                        nc.scalar.activation(ghn[:, c, :], gh_ps[:], ACT.Identity,
                                             bias=bhhn[:, c:c + 1])
                hnewT = gp.tile([128, 2, bl], F32)
                for c in range(2):
                    tmp = gp.tile([128, bl], F32, tag=f"tmp{c}")
                    nc.vector.tensor_mul(tmp[:], rz[:, c, :], ghn[:, c, :])
                    npre = gp.tile([128, bl], F32, tag=f"npre{c}")
                    nc.vector.tensor_add(npre[:], tmp[:], gin[:, c, :])
                    nsb = gp.tile([128, bl], F32, tag=f"nsb{c}")
                    nc.scalar.activation(nsb[:], npre[:], ACT.Tanh)
                    d = gp.tile([128, bl], F32, tag=f"d{c}")
                    nc.vector.tensor_sub(d[:], hidT[:, c, :], nsb[:])
                    e = gp.tile([128, bl], F32, tag=f"e{c}")
                    nc.vector.tensor_mul(e[:], rz[:, 2 + c, :], d[:])
                    nc.vector.tensor_add(hnewT[:, c, :], nsb[:], e[:])
                nc.scalar.dma_start(
                    hnewT_d.rearrange("(c p) b -> p c b", p=128), hnewT[:])
                # FC head
                fc_ps = gps.tile([bl, 1], F32, tag="fc")
                for kc in range(2):
                    nc.tensor.matmul(fc_ps[:], hnewT[:, kc, :], wfc[:, kc:kc + 1],
                                     start=(kc == 0), stop=(kc == 1))
                fcsb = gp.tile([bl, 1], F32)
                nc.scalar.activation(fcsb[:], fc_ps[:], ACT.Identity,
                                     bias=bfc[:, 0:1])
                nc.scalar.dma_start(fc_d[:], fcsb[:])

    nc.compile()
    return nc


_NC_CACHE = {}


def get_nc():
    key = (BL, S, G)
    if key not in _NC_CACHE:
        _NC_CACHE[key] = build_nc()
    return _NC_CACHE[key]


def make_in_maps(y_prev, hidden, enc_out, Wq, Wk, attn_b, Ws, bs,
                 W_ih, W_hh, b_ih, b_hh, Wfc, bfc):
    f32 = np.float32
    f16 = np.float16
    y_prev = np.asarray(y_prev, f32)
    hidden = np.asarray(hidden, f32)
    enc_out = np.asarray(enc_out, f32)
    shared = {
        "wkT": np.ascontiguousarray(np.asarray(Wk, f32).T).astype(f16),
        "wqT": np.ascontiguousarray(np.asarray(Wq, f32).T).astype(f16),
        "attnb": np.asarray(attn_b, f32).reshape(2, 128),
        "wsT": np.asarray(Ws, f32).reshape(2, 128).astype(f16),
        "wihT": np.ascontiguousarray(np.asarray(W_ih, f32).T).astype(f16),
        "whhT": np.ascontiguousarray(np.asarray(W_hh, f32).T).astype(f16),
        "bsum": (np.asarray(b_ih, f32)[:512] + np.asarray(b_hh, f32)[:512]
                 ).reshape(4, 128).astype(f32),
        "bihn": np.asarray(b_ih, f32)[512:].reshape(2, 128),
        "bhhn": np.asarray(b_hh, f32)[512:].reshape(2, 128),
        "wfc": np.asarray(Wfc, f32).reshape(2, 128),
        "bfc": np.full((BL, 1), np.asarray(bfc, f32).reshape(-1)[0], f32),
        "ident": np.eye(128, dtype=f16),
    }
    # bsum needs 6 rows in DRAM layout [6,128]; rows 4:6 unused by kernel
    shared["bsum"] = np.concatenate(
        [shared["bsum"], np.zeros((2, 128), f32)], axis=0)
    in_maps = []
    for i in range(NCORES):
        sl = slice(i * BL, (i + 1) * BL)
        m = dict(shared)
        m["enc"] = np.ascontiguousarray(enc_out[sl])
        m["ypT"] = np.ascontiguousarray(y_prev[sl, 0, :].T)
        m["hidT"] = np.ascontiguousarray(hidden[0, sl, :].T)
        in_maps.append(m)
    return in_maps


def kernel(**inputs):
    from concourse.bass_utils import run_bass_kernel_spmd
    nc = get_nc()
    in_maps = make_in_maps(**inputs)
    res = run_bass_kernel_spmd(nc, in_maps, list(range(NCORES))).results
    out = np.concatenate([r["fc"] for r in res], axis=0).astype(np.float32)
    h_new = np.concatenate([r["hnewT"].T for r in res], axis=0)[None]
    return out, h_new.astype(np.float32)


if __name__ == "__main__":
    # quick sim check on one core's shard
    from concourse.bass_interp import CoreSim
    rng = np.random.default_rng(0)
    sc = 1.0 / np.sqrt(H)
    inputs = {
        "y_prev": rng.standard_normal((B, 1, H), np.float32),
        "hidden": rng.standard_normal((1, B, H), np.float32),
        "enc_out": rng.standard_normal((B, S, H), np.float32),
        "Wq": rng.standard_normal((H, H), np.float32) * sc,
        "Wk": rng.standard_normal((H, H), np.float32) * sc,
        "attn_b": rng.uniform(0.1, 1.0, H).astype(np.float32),
        "Ws": rng.standard_normal((1, H), np.float32) * sc,
        "bs": rng.standard_normal((1,), np.float32) * sc,
        "W_ih": rng.standard_normal((3 * H, H), np.float32) * sc,
        "W_hh": rng.standard_normal((3 * H, H), np.float32) * sc,
        "b_ih": rng.standard_normal((3 * H,), np.float32) * sc,
        "b_hh": rng.standard_normal((3 * H,), np.float32) * sc,
        "Wfc": rng.standard_normal((1, H), np.float32) * sc,
        "bfc": rng.standard_normal((1,), np.float32) * sc,
    }
    nc = get_nc()
    print("built ok; instructions:", sum(1 for _ in nc.m.functions[0].instructions)
          if hasattr(nc.m.functions[0], "instructions") else "?")
    in_maps = make_in_maps(**inputs)
    sim = CoreSim(nc)
    for k, v in in_maps[0].items():
        sim.tensor(k)[:] = v
    sim.simulate()
    hnewT = np.array(sim.tensor("hnewT"))
    fc = np.array(sim.tensor("fc"))
    # numpy reference for core 0 shard
    e = inputs["enc_out"][:BL].astype(np.float64)
    hid = inputs["hidden"][0][:BL].astype(np.float64)
    q = hid @ inputs["Wq"].T.astype(np.float64)
    k_ = e @ inputs["Wk"].T.astype(np.float64)
    scs = np.tanh(k_ + q[:, None, :] + inputs["attn_b"]) @ inputs["Ws"][0].astype(np.float64)
    a = np.exp(scs - scs.max(-1, keepdims=True))
    a /= a.sum(-1, keepdims=True)
    ctx = np.einsum("bs,bsh->bh", a, e)
    x = ctx + inputs["y_prev"][:BL, 0, :]
    gi = x @ inputs["W_ih"].T.astype(np.float64) + inputs["b_ih"]
    gh = hid @ inputs["W_hh"].T.astype(np.float64) + inputs["b_hh"]
    i_r, i_z, i_n = np.split(gi, 3, -1)
    h_r, h_z, h_n = np.split(gh, 3, -1)
    r = 1 / (1 + np.exp(-(i_r + h_r)))
    z = 1 / (1 + np.exp(-(i_z + h_z)))
    n = np.tanh(i_n + r * h_n)
    h_new = (1 - z) * n + z * hid
    outr = h_new @ inputs["Wfc"][0].astype(np.float64) + inputs["bfc"][0]
    err_h = np.abs(hnewT.T - h_new) / (np.abs(h_new) + 1e-3)
    err_o = np.abs(fc[:, 0] - outr) / (np.abs(outr) + 1e-3)
    print("h_new rel err:", err_h.max(), "fc rel err:", err_o.max())


# revision 18
# speedup vs baseline: 71.0162x; 71.0162x over previous
"""Trainium2 Bass kernel for AttentionGRUDecoder (Bahdanau attention + GRU cell + FC head).

B=256, S=2048, H=256, f32 inputs. Data-parallel over batch across 8 NeuronCores
(32 batches/core). Weights replicated; small weights pre-transposed on host.

Per-core dataflow (per batch b):
  enc[b] [2048,256] f32  --DMA-->  SBUF natural (s = p*16+t permutation, benign)
     --DVE cast--> fp16  --xbar DMA transpose--> encT [h,s] fp16
  kT = WkT.T @ encT (PE, fp16)  --ACT tanh(+q+attn_b bias)--> t [h',s] fp16
  score = WsT.T @ t (PE) -> PSUM rows [G,2048] (one row per batch)
  softmax rows (DVE max / ACT exp+accum / DVE recip+scale)
  context^T[:,b] = sum_s encT * attn  (DVE scalar_tensor_tensor accum_out)
  GRU + FC on [h,b]-transposed layout (PE + DVE + ACT), outputs [H,BL] + [BL,1]
"""

import os
import sys

import numpy as np

sys.path.insert(0, "/opt/trn_rl_repo")

import concourse.bass as bass
import concourse.bacc as bacc
import concourse.mybir as mybir
import concourse.tile as tile

F32 = mybir.dt.float32
F16 = mybir.dt.float16
AX = mybir.AxisListType
ALU = mybir.AluOpType
ACT = mybir.ActivationFunctionType

B, S, H = 256, 2048, 256
NCORES = 8
BL = B // NCORES  # 32 batches per core
G = 8             # batches per softmax/context group (SBUF residency)

# Set False to materialize attn rows via gpsimd if broadcast-view APs fail.
BCAST_VIEW = False


def build_nc(bl=BL, s=S, g=G):
    """Build the per-core Bass program (SPMD: same program, sharded inputs)."""
    nc = bacc.Bacc("TRN2", target_bir_lowering=False, debug=False)
    st = s // 128   # number of 128-row s-tiles (16)
    ngrp = bl // g

    # ---- DRAM I/O ----
    enc_d = nc.declare_dram_parameter("enc", [bl, s, H], F32, isOutput=False)
    ypT_d = nc.declare_dram_parameter("ypT", [H, bl], F32, isOutput=False)
    hidT_d = nc.declare_dram_parameter("hidT", [H, bl], F32, isOutput=False)
    wkT_d = nc.declare_dram_parameter("wkT", [H, H], F16, isOutput=False)
    wqT_d = nc.declare_dram_parameter("wqT", [H, H], F16, isOutput=False)
    attnb_d = nc.declare_dram_parameter("attnb", [2, 128], F32, isOutput=False)
    wsT_d = nc.declare_dram_parameter("wsT", [2, 128], F16, isOutput=False)
    wihT_d = nc.declare_dram_parameter("wihT", [H, 3 * H], F16, isOutput=False)
    whhT_d = nc.declare_dram_parameter("whhT", [H, 3 * H], F16, isOutput=False)
    bsum_d = nc.declare_dram_parameter("bsum", [6, 128], F32, isOutput=False)
    bihn_d = nc.declare_dram_parameter("bihn", [2, 128], F32, isOutput=False)
    bhhn_d = nc.declare_dram_parameter("bhhn", [2, 128], F32, isOutput=False)
    wfc_d = nc.declare_dram_parameter("wfc", [2, 128], F32, isOutput=False)
    bfc_d = nc.declare_dram_parameter("bfc", [bl, 1], F32, isOutput=False)
    ident_d = nc.declare_dram_parameter("ident", [128, 128], F16, isOutput=False)
    hnewT_d = nc.declare_dram_parameter("hnewT", [H, bl], F32, isOutput=True)
    fc_d = nc.declare_dram_parameter("fc", [bl, 1], F32, isOutput=True)

    with tile.TileContext(nc) as tc:
        with tc.tile_pool(name="const", bufs=1) as cp:
            # ---- constants into SBUF ----
            wkT = cp.tile([128, 2, H], F16)      # [k-part, kc, h'] lhsT for kT matmul
            nc.scalar.dma_start(wkT[:], wkT_d.rearrange("(kc p) m -> p kc m", p=128))
            wqT = cp.tile([128, 2, H], F16)
            nc.scalar.dma_start(wqT[:], wqT_d.rearrange("(kc p) m -> p kc m", p=128))
            attnb = cp.tile([128, 2], F32)
            nc.scalar.dma_start(attnb[:], attnb_d.rearrange("c p -> p c"))
            wsT = cp.tile([128, 2], F16)
            nc.scalar.dma_start(wsT[:], wsT_d.rearrange("c p -> p c"))
            wihT = cp.tile([128, 2, 3 * H], F16)
            nc.scalar.dma_start(wihT[:], wihT_d.rearrange("(kc p) m -> p kc m", p=128))
            whhT = cp.tile([128, 2, 3 * H], F16)
            nc.scalar.dma_start(whhT[:], whhT_d.rearrange("(kc p) m -> p kc m", p=128))
            bsum = cp.tile([128, 6], F32)
            nc.scalar.dma_start(bsum[:], bsum_d.rearrange("c p -> p c"))
            bihn = cp.tile([128, 2], F32)
            nc.scalar.dma_start(bihn[:], bihn_d.rearrange("c p -> p c"))
            bhhn = cp.tile([128, 2], F32)
            nc.scalar.dma_start(bhhn[:], bhhn_d.rearrange("c p -> p c"))
            wfc = cp.tile([128, 2], F32)
            nc.scalar.dma_start(wfc[:], wfc_d.rearrange("c p -> p c"))
            bfc = cp.tile([bl, 1], F32)
            nc.scalar.dma_start(bfc[:], bfc_d[:])
            ypT = cp.tile([128, 2, bl], F32)
            nc.scalar.dma_start(ypT[:], ypT_d.rearrange("(c p) b -> p c b", p=128))
            hidT = cp.tile([128, 2, bl], F32)
            nc.scalar.dma_start(hidT[:], hidT_d.rearrange("(c p) b -> p c b", p=128))

            ident = cp.tile([128, 128], F16)
            nc.scalar.dma_start(ident[:], ident_d[:])
            hidT16 = cp.tile([128, 2, bl], F16)
            nc.vector.tensor_copy(hidT16[:], hidT[:])

            # bias_qb[h', c, b] = q^T + attn_b  (tanh bias, per-partition cols)
            bias_qb = cp.tile([128, 2, bl], F32)
            with tc.tile_pool(name="qpsum", bufs=2, space="PSUM") as qp:
                for c in range(2):
                    q_ps = qp.tile([128, bl], F32)
                    for kc in range(2):
                        nc.tensor.matmul(
                            q_ps[:],
                            wqT[:, kc, c * 128:(c + 1) * 128],
                            hidT16[:, kc, :],
                            start=(kc == 0), stop=(kc == 1),
                        )
                    nc.scalar.activation(
                        bias_qb[:, c, :], q_ps[:], ACT.Identity,
                        bias=attnb[:, c:c + 1],
                    )

            # context^T accumulator: [h-part, c*bl+b]
            ctxT = cp.tile([128, 2 * bl], F32)

            # ---- attention over batches ----
            with (
                tc.tile_pool(name="encf32", bufs=2) as pf32,
                tc.tile_pool(name="encf16", bufs=2) as pf16,
                tc.tile_pool(name="encT", bufs=g) as pT,
                tc.tile_pool(name="tpool", bufs=2) as pt,
                tc.tile_pool(name="attn", bufs=1) as pa,
                tc.tile_pool(name="scr", bufs=1) as psc,
                tc.tile_pool(name="stg", bufs=2) as pstg,
                tc.tile_pool(name="abc", bufs=2) as pabc,
                tc.tile_pool(name="ktpsum", bufs=2, space="PSUM") as kt_pool,
                tc.tile_pool(name="scpsum", bufs=1, space="PSUM") as sc_pool,
                tc.tile_pool(name="tppsum", bufs=2, space="PSUM") as tp_pool,
                tc.tile_pool(name="scdram", bufs=1, space="DRAM") as sc_dram,
            ):
                scd = sc_dram.tile([bl, s], F32)
                attnd = sc_dram.tile([bl, s], F16)
                for grp in range(ngrp):
                    encT_list = []
                    for j in range(g):
                        b = grp * g + j
                        # load natural f32, cast to f16 on DVE
                        # (s = p*st + t permutation, benign)
                        ef32 = pf32.tile([128, st, H], F32, tag="ef32")
                        nc.sync.dma_start(
                            ef32[:], enc_d[b].rearrange("(p t) h -> p t h", t=st))
                        ef16 = pf16.tile([128, st, H], F16, tag="ef16")
                        nc.vector.tensor_copy(ef16[:], ef32[:])
                        # PE transpose (identity matmul) -> PSUM -> copyback
                        eT = pT.tile([128, 2, s], F16, tag="encT")
                        encT_list.append(eT)
                        for tq in range(st // 4):
                            for c in range(2):
                                tp_ps = tp_pool.tile([128, 512], F16, tag="tp")
                                for i in range(4):
                                    t = tq * 4 + i
                                    nc.tensor.transpose(
                                        tp_ps[:, i * 128:(i + 1) * 128],
                                        ef16[:, t, c * 128:(c + 1) * 128],
                                        ident[:],
                                    )
                                eng = nc.vector.tensor_copy if (tq + c) % 2 else nc.scalar.copy
                                eng(eT[:, c, tq * 512:(tq + 1) * 512], tp_ps[:])
                        # kT matmul + tanh -> t tile
                        tt = pt.tile([128, 2, s], F16, tag="tt")
                        for c in range(2):
                            for q in range(s // 1024):
                                kt = kt_pool.tile([128, 1024], F32, tag="kt")
                                for qq in range(2):
                                    sl = slice(q * 1024 + qq * 512,
                                               q * 1024 + (qq + 1) * 512)
                                    for kc in range(2):
                                        nc.tensor.matmul(
                                            kt[:, qq * 512:(qq + 1) * 512],
                                            wkT[:, kc, c * 128:(c + 1) * 128],
                                            eT[:, kc, sl],
                                            start=(kc == 0), stop=(kc == 1),
                                        )
                                nc.scalar.activation(
                                    tt[:, c, q * 1024:(q + 1) * 1024], kt[:],
                                    ACT.Tanh, bias=bias_qb[:, c, b:b + 1],
                                )
                        # score row: Ws^T @ t -> psum [1,1024] halves -> SBUF -> DRAM
                        stg = pstg.tile([1, s], F32, tag="stg")
                        for half in range(2):
                            sc_ps = sc_pool.tile([1, 1024], F32, tag="scps")
                            for qq in range(2):
                                sl_p = slice(qq * 512, (qq + 1) * 512)
                                sl_t = slice(half * 1024 + qq * 512,
                                             half * 1024 + (qq + 1) * 512)
                                for kc in range(2):
                                    nc.tensor.matmul(
                                        sc_ps[0:1, sl_p],
                                        wsT[:, kc:kc + 1],
                                        tt[:, kc, sl_t],
                                        start=(kc == 0), stop=(kc == 1),
                                    )
                            nc.scalar.activation(
                                stg[0:1, half * 1024:(half + 1) * 1024],
                                sc_ps[:], ACT.Copy)
                        nc.scalar.dma_start(scd[b:b + 1, :], stg[:])
                    # ---- group softmax ----
                    scg = pa.tile([g, s], F32, tag="scg")
                    nc.scalar.dma_start(scg[:], scd[grp * g:(grp + 1) * g, :])
                    mx = pa.tile([g, 1], F32, tag="mx")
                    nc.vector.reduce_max(mx[:], scg[:], axis=AX.X)
                    nmx = pa.tile([g, 1], F32, tag="nmx")
                    nc.vector.tensor_scalar_mul(nmx[:], mx[:], -1.0)
                    expt = pa.tile([g, s], F16, tag="expt")
                    sume = pa.tile([g, 1], F32, tag="sume")
                    nc.scalar.activation(expt[:], scg[:], ACT.Exp,
                                         bias=nmx[:], accum_out=sume[:])
                    inv = pa.tile([g, 1], F32, tag="inv")
                    nc.vector.reciprocal(inv[:], sume[:])
                    attn = pa.tile([g, s], F16, tag="attn")
                    nc.vector.tensor_scalar_mul(attn[:], expt[:], inv[:])
                    nc.scalar.dma_start(attnd[grp * g:(grp + 1) * g, :], attn[:])
                    # ---- context per batch ----
                    for j in range(g):
                        b = grp * g + j
                        for c in range(2):
                            scr = psc.tile([128, s], F16, tag="scr")
                            if c == 0:
                                a_bc = pabc.tile([128, s], F16, tag="abc")
                                nc.scalar.dma_start(
                                    a_bc[:], attnd[b:b + 1, :].broadcast_to([128, s]))
                            a_row = a_bc[:]
                            nc.vector.scalar_tensor_tensor(
                                scr[:], encT_list[j][:, c, :], 1.0, a_row,
                                op0=ALU.mult, op1=ALU.mult,
                                accum_out=ctxT[:, c * bl + b:c * bl + b + 1],
                            )

            # ---- GRU cell + FC head (transposed [h, b] layout) ----
            with (
                tc.tile_pool(name="gru", bufs=1) as gp,
                tc.tile_pool(name="grupsum", bufs=2, space="PSUM") as gps,
            ):
                xT16 = gp.tile([128, 2, bl], F16)
                for c in range(2):
                    nc.vector.tensor_add(xT16[:, c, :],
                                         ctxT[:, c * bl:(c + 1) * bl],
                                         ypT[:, c, :])
                rz = gp.tile([128, 4, bl], F32)
                gin = gp.tile([128, 2, bl], F32)
                ghn = gp.tile([128, 2, bl], F32)
                for c6 in range(6):
                    if c6 < 4:
                        # gi+gh accumulate in one PSUM group (4 matmuls)
                        g_ps = gps.tile([128, bl], F32, tag="gi")
                        for kc in range(2):
                            nc.tensor.matmul(g_ps[:],
                                             wihT[:, kc, c6 * 128:(c6 + 1) * 128],
                                             xT16[:, kc, :],
                                             start=(kc == 0), stop=False)
                        for kc in range(2):
                            nc.tensor.matmul(g_ps[:],
                                             whhT[:, kc, c6 * 128:(c6 + 1) * 128],
                                             hidT16[:, kc, :],
                                             start=False, stop=(kc == 1))
                        nc.scalar.activation(rz[:, c6, :], g_ps[:], ACT.Sigmoid,
                                             bias=bsum[:, c6:c6 + 1])
                    else:
                        gi_ps = gps.tile([128, bl], F32, tag="gi")
                        gh_ps = gps.tile([128, bl], F32, tag="gh")
                        for kc in range(2):
                            nc.tensor.matmul(gi_ps[:],
                                             wihT[:, kc, c6 * 128:(c6 + 1) * 128],
                                             xT16[:, kc, :],
                                             start=(kc == 0), stop=(kc == 1))
                        for kc in range(2):
                            nc.tensor.matmul(gh_ps[:],
                                             whhT[:, kc, c6 * 128:(c6 + 1) * 128],
                                             hidT16[:, kc, :],
                                             start=(kc == 0), stop=(kc == 1))
                        c = c6 - 4
                        nc.scalar.activation(gin[:, c, :], gi_ps[:], ACT.Identity,
                                             bias=bihn[:, c:c + 1])
                        nc.scalar.activation(ghn[:, c, :], gh_ps[:], ACT.Identity,
                                             bias=bhhn[:, c:c + 1])
                hnewT = gp.tile([128, 2, bl], F32)
                for c in range(2):
                    tmp = gp.tile([128, bl], F32, tag=f"tmp{c}")
                    nc.vector.tensor_mul(tmp[:], rz[:, c, :], ghn[:, c, :])
                    npre = gp.tile([128, bl], F32, tag=f"npre{c}")
                    nc.vector.tensor_add(npre[:], tmp[:], gin[:, c, :])
                    nsb = gp.tile([128, bl], F32, tag=f"nsb{c}")
                    nc.scalar.activation(nsb[:], npre[:], ACT.Tanh)
                    d = gp.tile([128, bl], F32, tag=f"d{c}")
                    nc.vector.tensor_sub(d[:], hidT[:, c, :], nsb[:])
                    e = gp.tile([128, bl], F32, tag=f"e{c}")
                    nc.vector.tensor_mul(e[:], rz[:, 2 + c, :], d[:])
                    nc.vector.tensor_add(hnewT[:, c, :], nsb[:], e[:])
                nc.scalar.dma_start(
                    hnewT_d.rearrange("(c p) b -> p c b", p=128), hnewT[:])
                # FC head
                fc_ps = gps.tile([bl, 1], F32, tag="fc")
                for kc in range(2):
                    nc.tensor.matmul(fc_ps[:], hnewT[:, kc, :], wfc[:, kc:kc + 1],
                                     start=(kc == 0), stop=(kc == 1))
                fcsb = gp.tile([bl, 1], F32)
                nc.scalar.activation(fcsb[:], fc_ps[:], ACT.Identity,
                                     bias=bfc[:, 0:1])
                nc.scalar.dma_start(fc_d[:], fcsb[:])

    nc.compile()
    return nc


_NC_CACHE = {}


def get_nc():
    key = (BL, S, G)
    if key not in _NC_CACHE:
        _NC_CACHE[key] = build_nc()
    return _NC_CACHE[key]


def make_in_maps(y_prev, hidden, enc_out, Wq, Wk, attn_b, Ws, bs,
                 W_ih, W_hh, b_ih, b_hh, Wfc, bfc):
    f32 = np.float32
    f16 = np.float16
    y_prev = np.asarray(y_prev, f32)
    hidden = np.asarray(hidden, f32)
    enc_out = np.asarray(enc_out, f32)
    shared = {
        "wkT": np.ascontiguousarray(np.asarray(Wk, f32).T).astype(f16),
        "wqT": np.ascontiguousarray(np.asarray(Wq, f32).T).astype(f16),
        "attnb": np.asarray(attn_b, f32).reshape(2, 128),
        "wsT": np.asarray(Ws, f32).reshape(2, 128).astype(f16),
        "wihT": np.ascontiguousarray(np.asarray(W_ih, f32).T).astype(f16),
        "whhT": np.ascontiguousarray(np.asarray(W_hh, f32).T).astype(f16),
        "bsum": (np.asarray(b_ih, f32)[:512] + np.asarray(b_hh, f32)[:512]
                 ).reshape(4, 128).astype(f32),
        "bihn": np.asarray(b_ih, f32)[512:].reshape(2, 128),
        "bhhn": np.asarray(b_hh, f32)[512:].reshape(2, 128),
        "wfc": np.asarray(Wfc, f32).reshape(2, 128),
        "bfc": np.full((BL, 1), np.asarray(bfc, f32).reshape(-1)[0], f32),
        "ident": np.eye(128, dtype=f16),
    }
    # bsum needs 6 rows in DRAM layout [6,128]; rows 4:6 unused by kernel
    shared["bsum"] = np.concatenate(
        [shared["bsum"], np.zeros((2, 128), f32)], axis=0)
    in_maps = []
    for i in range(NCORES):
        sl = slice(i * BL, (i + 1) * BL)
        m = dict(shared)
        m["enc"] = np.ascontiguousarray(enc_out[sl])
        m["ypT"] = np.ascontiguousarray(y_prev[sl, 0, :].T)
        m["hidT"] = np.ascontiguousarray(hidden[0, sl, :].T)
        in_maps.append(m)
    return in_maps


def kernel(**inputs):
    from concourse.bass_utils import run_bass_kernel_spmd
    nc = get_nc()
    in_maps = make_in_maps(**inputs)
    res = run_bass_kernel_spmd(nc, in_maps, list(range(NCORES))).results
    out = np.concatenate([r["fc"] for r in res], axis=0).astype(np.float32)
    h_new = np.concatenate([r["hnewT"].T for r in res], axis=0)[None]
    return out, h_new.astype(np.float32)


if __name__ == "__main__":
    # quick sim check on one core's shard
    from concourse.bass_interp import CoreSim
    rng = np.random.default_rng(0)
    sc = 1.0 / np.sqrt(H)
    inputs = {
        "y_prev": rng.standard_normal((B, 1, H), np.float32),
        "hidden": rng.standard_normal((1, B, H), np.float32),
        "enc_out": rng.standard_normal((B, S, H), np.float32),
        "Wq": rng.standard_normal((H, H), np.float32) * sc,
        "Wk": rng.standard_normal((H, H), np.float32) * sc,
        "attn_b": rng.uniform(0.1, 1.0, H).astype(np.float32),
        "Ws": rng.standard_normal((1, H), np.float32) * sc,
        "bs": rng.standard_normal((1,), np.float32) * sc,
        "W_ih": rng.standard_normal((3 * H, H), np.float32) * sc,
        "W_hh": rng.standard_normal((3 * H, H), np.float32) * sc,
        "b_ih": rng.standard_normal((3 * H,), np.float32) * sc,
        "b_hh": rng.standard_normal((3 * H,), np.float32) * sc,
        "Wfc": rng.standard_normal((1, H), np.float32) * sc,
        "bfc": rng.standard_normal((1,), np.float32) * sc,
    }
    nc = get_nc()
    print("built ok; instructions:", sum(1 for _ in nc.m.functions[0].instructions)
          if hasattr(nc.m.functions[0], "instructions") else "?")
    in_maps = make_in_maps(**inputs)
    sim = CoreSim(nc)
    for k, v in in_maps[0].items():
        sim.tensor(k)[:] = v
    sim.simulate()
    hnewT = np.array(sim.tensor("hnewT"))
    fc = np.array(sim.tensor("fc"))
    # numpy reference for core 0 shard
    e = inputs["enc_out"][:BL].astype(np.float64)
    hid = inputs["hidden"][0][:BL].astype(np.float64)
    q = hid @ inputs["Wq"].T.astype(np.float64)
    k_ = e @ inputs["Wk"].T.astype(np.float64)
    scs = np.tanh(k_ + q[:, None, :] + inputs["attn_b"]) @ inputs["Ws"][0].astype(np.float64)
    a = np.exp(scs - scs.max(-1, keepdims=True))
    a /= a.sum(-1, keepdims=True)
    ctx = np.einsum("bs,bsh->bh", a, e)
    x = ctx + inputs["y_prev"][:BL, 0, :]
    gi = x @ inputs["W_ih"].T.astype(np.float64) + inputs["b_ih"]
    gh = hid @ inputs["W_hh"].T.astype(np.float64) + inputs["b_hh"]
    i_r, i_z, i_n = np.split(gi, 3, -1)
    h_r, h_z, h_n = np.split(gh, 3, -1)
    r = 1 / (1 + np.exp(-(i_r + h_r)))
    z = 1 / (1 + np.exp(-(i_z + h_z)))
    n = np.tanh(i_n + r * h_n)
    h_new = (1 - z) * n + z * hid
    outr = h_new @ inputs["Wfc"][0].astype(np.float64) + inputs["bfc"][0]
    err_h = np.abs(hnewT.T - h_new) / (np.abs(h_new) + 1e-3)
    err_o = np.abs(fc[:, 0] - outr) / (np.abs(outr) + 1e-3)
    print("h_new rel err:", err_h.max(), "fc rel err:", err_o.max())


# revision 19
# speedup vs baseline: 145.7382x; 2.0522x over previous
"""Trainium2 Bass kernel for AttentionGRUDecoder (Bahdanau attention + GRU cell + FC head).

B=256, S=2048, H=256, f32 inputs. Data-parallel over batch across 8 NeuronCores
(32 batches/core). Weights replicated; small weights pre-transposed on host.

Per-core dataflow (per batch b):
  enc[b] [2048,256] f32  --DMA-->  SBUF natural (s = p*16+t permutation, benign)
     --DVE cast--> fp16  --xbar DMA transpose--> encT [h,s] fp16
  kT = WkT.T @ encT (PE, fp16)  --ACT tanh(+q+attn_b bias)--> t [h',s] fp16
  score = WsT.T @ t (PE) -> PSUM rows [G,2048] (one row per batch)
  softmax rows (DVE max / ACT exp+accum / DVE recip+scale)
  context^T[:,b] = sum_s encT * attn  (DVE scalar_tensor_tensor accum_out)
  GRU + FC on [h,b]-transposed layout (PE + DVE + ACT), outputs [H,BL] + [BL,1]
"""

import os
import sys

import numpy as np

sys.path.insert(0, "/opt/trn_rl_repo")

import concourse.bass as bass
import concourse.bacc as bacc
import concourse.mybir as mybir
import concourse.tile as tile

F32 = mybir.dt.float32
F16 = mybir.dt.float16
AX = mybir.AxisListType
ALU = mybir.AluOpType
ACT = mybir.ActivationFunctionType

B, S, H = 256, 2048, 256
NCORES = 8
BL = B // NCORES  # 32 batches per core
G = 4             # batches per softmax/context group (SBUF residency)

# Set False to materialize attn rows via gpsimd if broadcast-view APs fail.
BCAST_VIEW = False


def build_nc(bl=BL, s=S, g=G):
    """Build the per-core Bass program (SPMD: same program, sharded inputs)."""
    nc = bacc.Bacc("TRN2", target_bir_lowering=False, debug=False)
    st = s // 128   # number of 128-row s-tiles (16)
    ngrp = bl // g

    # ---- DRAM I/O ----
    enc_d = nc.declare_dram_parameter("enc", [bl, s, H], F32, isOutput=False)
    ypT_d = nc.declare_dram_parameter("ypT", [H, bl], F32, isOutput=False)
    hidT_d = nc.declare_dram_parameter("hidT", [H, bl], F32, isOutput=False)
    wkT_d = nc.declare_dram_parameter("wkT", [H, H], F16, isOutput=False)
    wqT_d = nc.declare_dram_parameter("wqT", [H, H], F16, isOutput=False)
    attnb_d = nc.declare_dram_parameter("attnb", [2, 128], F32, isOutput=False)
    wsT_d = nc.declare_dram_parameter("wsT", [2, 128], F16, isOutput=False)
    wihT_d = nc.declare_dram_parameter("wihT", [H, 3 * H], F16, isOutput=False)
    whhT_d = nc.declare_dram_parameter("whhT", [H, 3 * H], F16, isOutput=False)
    bsum_d = nc.declare_dram_parameter("bsum", [6, 128], F32, isOutput=False)
    bihn_d = nc.declare_dram_parameter("bihn", [2, 128], F32, isOutput=False)
    bhhn_d = nc.declare_dram_parameter("bhhn", [2, 128], F32, isOutput=False)
    wfc_d = nc.declare_dram_parameter("wfc", [2, 128], F32, isOutput=False)
    bfc_d = nc.declare_dram_parameter("bfc", [bl, 1], F32, isOutput=False)
    ident_d = nc.declare_dram_parameter("ident", [128, 128], F16, isOutput=False)
    hnewT_d = nc.declare_dram_parameter("hnewT", [H, bl], F32, isOutput=True)
    fc_d = nc.declare_dram_parameter("fc", [bl, 1], F32, isOutput=True)

    with tile.TileContext(nc) as tc:
        with tc.tile_pool(name="const", bufs=1) as cp:
            # ---- constants into SBUF ----
            wkT = cp.tile([128, 2, H], F16)      # [k-part, kc, h'] lhsT for kT matmul
            nc.scalar.dma_start(wkT[:], wkT_d.rearrange("(kc p) m -> p kc m", p=128))
            wqT = cp.tile([128, 2, H], F16)
            nc.scalar.dma_start(wqT[:], wqT_d.rearrange("(kc p) m -> p kc m", p=128))
            attnb = cp.tile([128, 2], F32)
            nc.scalar.dma_start(attnb[:], attnb_d.rearrange("c p -> p c"))
            wsT = cp.tile([128, 2], F16)
            nc.scalar.dma_start(wsT[:], wsT_d.rearrange("c p -> p c"))
            wihT = cp.tile([128, 2, 3 * H], F16)
            nc.scalar.dma_start(wihT[:], wihT_d.rearrange("(kc p) m -> p kc m", p=128))
            whhT = cp.tile([128, 2, 3 * H], F16)
            nc.scalar.dma_start(whhT[:], whhT_d.rearrange("(kc p) m -> p kc m", p=128))
            bsum = cp.tile([128, 6], F32)
            nc.scalar.dma_start(bsum[:], bsum_d.rearrange("c p -> p c"))
            bihn = cp.tile([128, 2], F32)
            nc.scalar.dma_start(bihn[:], bihn_d.rearrange("c p -> p c"))
            bhhn = cp.tile([128, 2], F32)
            nc.scalar.dma_start(bhhn[:], bhhn_d.rearrange("c p -> p c"))
            wfc = cp.tile([128, 2], F32)
            nc.scalar.dma_start(wfc[:], wfc_d.rearrange("c p -> p c"))
            bfc = cp.tile([bl, 1], F32)
            nc.scalar.dma_start(bfc[:], bfc_d[:])
            ypT = cp.tile([128, 2, bl], F32)
            nc.scalar.dma_start(ypT[:], ypT_d.rearrange("(c p) b -> p c b", p=128))
            hidT = cp.tile([128, 2, bl], F32)
            nc.scalar.dma_start(hidT[:], hidT_d.rearrange("(c p) b -> p c b", p=128))

            ident = cp.tile([128, 128], F16)
            nc.scalar.dma_start(ident[:], ident_d[:])
            hidT16 = cp.tile([128, 2, bl], F16)
            nc.vector.tensor_copy(hidT16[:], hidT[:])

            # bias_qb[h', c, b] = q^T + attn_b  (tanh bias, per-partition cols)
            bias_qb = cp.tile([128, 2, bl], F32)
            with tc.tile_pool(name="qpsum", bufs=2, space="PSUM") as qp:
                for c in range(2):
                    q_ps = qp.tile([128, bl], F32)
                    for kc in range(2):
                        nc.tensor.matmul(
                            q_ps[:],
                            wqT[:, kc, c * 128:(c + 1) * 128],
                            hidT16[:, kc, :],
                            start=(kc == 0), stop=(kc == 1),
                        )
                    nc.scalar.activation(
                        bias_qb[:, c, :], q_ps[:], ACT.Identity,
                        bias=attnb[:, c:c + 1],
                    )

            # context^T accumulator: [h-part, c*bl+b]
            ctxT = cp.tile([128, 2 * bl], F32)

            # ---- attention over batches ----
            with (
                tc.tile_pool(name="encf32", bufs=2) as pf32,
                tc.tile_pool(name="encf16", bufs=2) as pf16,
                tc.tile_pool(name="encT", bufs=g) as pT,
                tc.tile_pool(name="tpool", bufs=2) as pt,
                tc.tile_pool(name="attn", bufs=2) as pa,
                tc.tile_pool(name="scr", bufs=1) as psc,
                tc.tile_pool(name="stg", bufs=2) as pstg,
                tc.tile_pool(name="abc", bufs=2) as pabc,
                tc.tile_pool(name="ktpsum", bufs=2, space="PSUM") as kt_pool,
                tc.tile_pool(name="scpsum", bufs=2, space="PSUM") as sc_pool,
                tc.tile_pool(name="scdram", bufs=1, space="DRAM") as sc_dram,
            ):
                scd = sc_dram.tile([bl, s], F32)
                attnd = sc_dram.tile([bl, s], F16)
                for grp in range(ngrp):
                    encT_list = []
                    for j in range(g):
                        b = grp * g + j
                        # load natural f32, cast to f16 on DVE
                        # (s = p*st + t permutation, benign)
                        ef32 = pf32.tile([128, st, H], F32, tag="ef32")
                        nc.sync.dma_start(
                            ef32[:], enc_d[b].rearrange("(p t) h -> p t h", t=st))
                        ef16 = pf16.tile([128, st, H], F16, tag="ef16")
                        nc.vector.tensor_copy(ef16[:], ef32[:])
                        # SBUF->SBUF xbar DMA transpose (compiles under Bacc)
                        eT = pT.tile([128, 2, s], F16, tag="encT")
                        encT_list.append(eT)
                        for t in range(st):
                            for c in range(2):
                                nc.sync.dma_start_transpose(
                                    eT[:, c, t * 128:(t + 1) * 128],
                                    ef16[:, t, c * 128:(c + 1) * 128],
                                )
                        # kT matmul + tanh -> t tile
                        tt = pt.tile([128, 2, s], F16, tag="tt")
                        for c in range(2):
                            for q in range(s // 1024):
                                kt = kt_pool.tile([128, 1024], F32, tag="kt")
                                for qq in range(2):
                                    sl = slice(q * 1024 + qq * 512,
                                               q * 1024 + (qq + 1) * 512)
                                    for kc in range(2):
                                        nc.tensor.matmul(
                                            kt[:, qq * 512:(qq + 1) * 512],
                                            wkT[:, kc, c * 128:(c + 1) * 128],
                                            eT[:, kc, sl],
                                            start=(kc == 0), stop=(kc == 1),
                                        )
                                nc.scalar.activation(
                                    tt[:, c, q * 1024:(q + 1) * 1024], kt[:],
                                    ACT.Tanh, bias=bias_qb[:, c, b:b + 1],
                                )
                        # score row: Ws^T @ t -> psum [1,1024] halves -> SBUF -> DRAM
                        stg = pstg.tile([1, s], F32, tag="stg")
                        for half in range(2):
                            sc_ps = sc_pool.tile([1, 1024], F32, tag="scps")
                            for qq in range(2):
                                sl_p = slice(qq * 512, (qq + 1) * 512)
                                sl_t = slice(half * 1024 + qq * 512,
                                             half * 1024 + (qq + 1) * 512)
                                for kc in range(2):
                                    nc.tensor.matmul(
                                        sc_ps[0:1, sl_p],
                                        wsT[:, kc:kc + 1],
                                        tt[:, kc, sl_t],
                                        start=(kc == 0), stop=(kc == 1),
                                    )
                            nc.scalar.activation(
                                stg[0:1, half * 1024:(half + 1) * 1024],
                                sc_ps[:], ACT.Copy)
                        nc.scalar.dma_start(scd[b:b + 1, :], stg[:])
                    # ---- group softmax ----
                    scg = pa.tile([g, s], F32, tag="scg")
                    nc.scalar.dma_start(scg[:], scd[grp * g:(grp + 1) * g, :])
                    mx = pa.tile([g, 1], F32, tag="mx")
                    nc.vector.reduce_max(mx[:], scg[:], axis=AX.X)
                    nmx = pa.tile([g, 1], F32, tag="nmx")
                    nc.vector.tensor_scalar_mul(nmx[:], mx[:], -1.0)
                    expt = pa.tile([g, s], F16, tag="expt")
                    sume = pa.tile([g, 1], F32, tag="sume")
                    nc.scalar.activation(expt[:], scg[:], ACT.Exp,
                                         bias=nmx[:], accum_out=sume[:])
                    inv = pa.tile([g, 1], F32, tag="inv")
                    nc.vector.reciprocal(inv[:], sume[:])
                    attn = pa.tile([g, s], F16, tag="attn")
                    nc.vector.tensor_scalar_mul(attn[:], expt[:], inv[:])
                    nc.scalar.dma_start(attnd[grp * g:(grp + 1) * g, :], attn[:])
                    # ---- context per batch ----
                    for j in range(g):
                        b = grp * g + j
                        for c in range(2):
                            scr = psc.tile([128, s], F16, tag="scr")
                            if c == 0:
                                a_bc = pabc.tile([128, s], F16, tag="abc")
                                nc.scalar.dma_start(
                                    a_bc[:], attnd[b:b + 1, :].broadcast_to([128, s]))
                            a_row = a_bc[:]
                            nc.vector.scalar_tensor_tensor(
                                scr[:], encT_list[j][:, c, :], 1.0, a_row,
                                op0=ALU.mult, op1=ALU.mult,
                                accum_out=ctxT[:, c * bl + b:c * bl + b + 1],
                            )

            # ---- GRU cell + FC head (transposed [h, b] layout) ----
            with (
                tc.tile_pool(name="gru", bufs=1) as gp,
                tc.tile_pool(name="grupsum", bufs=2, space="PSUM") as gps,
            ):
                xT16 = gp.tile([128, 2, bl], F16)
                for c in range(2):
                    nc.vector.tensor_add(xT16[:, c, :],
                                         ctxT[:, c * bl:(c + 1) * bl],
                                         ypT[:, c, :])
                rz = gp.tile([128, 4, bl], F32)
                gin = gp.tile([128, 2, bl], F32)
                ghn = gp.tile([128, 2, bl], F32)
                for c6 in range(6):
                    if c6 < 4:
                        # gi+gh accumulate in one PSUM group (4 matmuls)
                        g_ps = gps.tile([128, bl], F32, tag="gi")
                        for kc in range(2):
                            nc.tensor.matmul(g_ps[:],
                                             wihT[:, kc, c6 * 128:(c6 + 1) * 128],
                                             xT16[:, kc, :],
                                             start=(kc == 0), stop=False)
                        for kc in range(2):
                            nc.tensor.matmul(g_ps[:],
                                             whhT[:, kc, c6 * 128:(c6 + 1) * 128],
                                             hidT16[:, kc, :],
                                             start=False, stop=(kc == 1))
                        nc.scalar.activation(rz[:, c6, :], g_ps[:], ACT.Sigmoid,
                                             bias=bsum[:, c6:c6 + 1])
                    else:
                        gi_ps = gps.tile([128, bl], F32, tag="gi")
                        gh_ps = gps.tile([128, bl], F32, tag="gh")
                        for kc in range(2):
                            nc.tensor.matmul(gi_ps[:],
                                             wihT[:, kc, c6 * 128:(c6 + 1) * 128],
                                             xT16[:, kc, :],
                                             start=(kc == 0), stop=(kc == 1))
                        for kc in range(2):
                            nc.tensor.matmul(gh_ps[:],
                                             whhT[:, kc, c6 * 128:(c6 + 1) * 128],
                                             hidT16[:, kc, :],
                                             start=(kc == 0), stop=(kc == 1))
                        c = c6 - 4
                        nc.scalar.activation(gin[:, c, :], gi_ps[:], ACT.Identity,
                                             bias=bihn[:, c:c + 1])
                        nc.scalar.activation(ghn[:, c, :], gh_ps[:], ACT.Identity,
                                             bias=bhhn[:, c:c + 1])
                hnewT = gp.tile([128, 2, bl], F32)
                for c in range(2):
                    tmp = gp.tile([128, bl], F32, tag=f"tmp{c}")
                    nc.vector.tensor_mul(tmp[:], rz[:, c, :], ghn[:, c, :])
                    npre = gp.tile([128, bl], F32, tag=f"npre{c}")
                    nc.vector.tensor_add(npre[:], tmp[:], gin[:, c, :])
                    nsb = gp.tile([128, bl], F32, tag=f"nsb{c}")
                    nc.scalar.activation(nsb[:], npre[:], ACT.Tanh)
                    d = gp.tile([128, bl], F32, tag=f"d{c}")
                    nc.vector.tensor_sub(d[:], hidT[:, c, :], nsb[:])
                    e = gp.tile([128, bl], F32, tag=f"e{c}")
                    nc.vector.tensor_mul(e[:], rz[:, 2 + c, :], d[:])
                    nc.vector.tensor_add(hnewT[:, c, :], nsb[:], e[:])
                nc.scalar.dma_start(
                    hnewT_d.rearrange("(c p) b -> p c b", p=128), hnewT[:])
                # FC head
                fc_ps = gps.tile([bl, 1], F32, tag="fc")
                for kc in range(2):
                    nc.tensor.matmul(fc_ps[:], hnewT[:, kc, :], wfc[:, kc:kc + 1],
                                     start=(kc == 0), stop=(kc == 1))
                fcsb = gp.tile([bl, 1], F32)
                nc.scalar.activation(fcsb[:], fc_ps[:], ACT.Identity,
                                     bias=bfc[:, 0:1])
                nc.scalar.dma_start(fc_d[:], fcsb[:])

    nc.compile()
    return nc


_NC_CACHE = {}


def get_nc():
    key = (BL, S, G)
    if key not in _NC_CACHE:
        _NC_CACHE[key] = build_nc()
    return _NC_CACHE[key]


def make_in_maps(y_prev, hidden, enc_out, Wq, Wk, attn_b, Ws, bs,
                 W_ih, W_hh, b_ih, b_hh, Wfc, bfc):
    f32 = np.float32
    f16 = np.float16
    y_prev = np.asarray(y_prev, f32)
    hidden = np.asarray(hidden, f32)
    enc_out = np.asarray(enc_out, f32)
    shared = {
        "wkT": np.ascontiguousarray(np.asarray(Wk, f32).T).astype(f16),
        "wqT": np.ascontiguousarray(np.asarray(Wq, f32).T).astype(f16),
        "attnb": np.asarray(attn_b, f32).reshape(2, 128),
        "wsT": np.asarray(Ws, f32).reshape(2, 128).astype(f16),
        "wihT": np.ascontiguousarray(np.asarray(W_ih, f32).T).astype(f16),
        "whhT": np.ascontiguousarray(np.asarray(W_hh, f32).T).astype(f16),
        "bsum": (np.asarray(b_ih, f32)[:512] + np.asarray(b_hh, f32)[:512]
                 ).reshape(4, 128).astype(f32),
        "bihn": np.asarray(b_ih, f32)[512:].reshape(2, 128),
        "bhhn": np.asarray(b_hh, f32)[512:].reshape(2, 128),
        "wfc": np.asarray(Wfc, f32).reshape(2, 128),
        "bfc": np.full((BL, 1), np.asarray(bfc, f32).reshape(-1)[0], f32),
        "ident": np.eye(128, dtype=f16),
    }
    # bsum needs 6 rows in DRAM layout [6,128]; rows 4:6 unused by kernel
    shared["bsum"] = np.concatenate(
        [shared["bsum"], np.zeros((2, 128), f32)], axis=0)
    in_maps = []
    for i in range(NCORES):
        sl = slice(i * BL, (i + 1) * BL)
        m = dict(shared)
        m["enc"] = np.ascontiguousarray(enc_out[sl])
        m["ypT"] = np.ascontiguousarray(y_prev[sl, 0, :].T)
        m["hidT"] = np.ascontiguousarray(hidden[0, sl, :].T)
        in_maps.append(m)
    return in_maps


def kernel(**inputs):
    from concourse.bass_utils import run_bass_kernel_spmd
    nc = get_nc()
    in_maps = make_in_maps(**inputs)
    res = run_bass_kernel_spmd(nc, in_maps, list(range(NCORES))).results
    out = np.concatenate([r["fc"] for r in res], axis=0).astype(np.float32)
    h_new = np.concatenate([r["hnewT"].T for r in res], axis=0)[None]
    return out, h_new.astype(np.float32)


if __name__ == "__main__":
    # quick sim check on one core's shard
    from concourse.bass_interp import CoreSim
    rng = np.random.default_rng(0)
    sc = 1.0 / np.sqrt(H)
    inputs = {
        "y_prev": rng.standard_normal((B, 1, H), np.float32),
        "hidden": rng.standard_normal((1, B, H), np.float32),
        "enc_out": rng.standard_normal((B, S, H), np.float32),
        "Wq": rng.standard_normal((H, H), np.float32) * sc,
        "Wk": rng.standard_normal((H, H), np.float32) * sc,
        "attn_b": rng.uniform(0.1, 1.0, H).astype(np.float32),
        "Ws": rng.standard_normal((1, H), np.float32) * sc,
        "bs": rng.standard_normal((1,), np.float32) * sc,
        "W_ih": rng.standard_normal((3 * H, H), np.float32) * sc,
        "W_hh": rng.standard_normal((3 * H, H), np.float32) * sc,
        "b_ih": rng.standard_normal((3 * H,), np.float32) * sc,
        "b_hh": rng.standard_normal((3 * H,), np.float32) * sc,
        "Wfc": rng.standard_normal((1, H), np.float32) * sc,
        "bfc": rng.standard_normal((1,), np.float32) * sc,
    }
    nc = get_nc()
    print("built ok; instructions:", sum(1 for _ in nc.m.functions[0].instructions)
          if hasattr(nc.m.functions[0], "instructions") else "?")
    in_maps = make_in_maps(**inputs)
    sim = CoreSim(nc)
    for k, v in in_maps[0].items():
        sim.tensor(k)[:] = v
    sim.simulate()
    hnewT = np.array(sim.tensor("hnewT"))
    fc = np.array(sim.tensor("fc"))
    # numpy reference for core 0 shard
    e = inputs["enc_out"][:BL].astype(np.float64)
    hid = inputs["hidden"][0][:BL].astype(np.float64)
    q = hid @ inputs["Wq"].T.astype(np.float64)
    k_ = e @ inputs["Wk"].T.astype(np.float64)
    scs = np.tanh(k_ + q[:, None, :] + inputs["attn_b"]) @ inputs["Ws"][0].astype(np.float64)
    a = np.exp(scs - scs.max(-1, keepdims=True))
    a /= a.sum(-1, keepdims=True)
    ctx = np.einsum("bs,bsh->bh", a, e)
    x = ctx + inputs["y_prev"][:BL, 0, :]
    gi = x @ inputs["W_ih"].T.astype(np.float64) + inputs["b_ih"]
    gh = hid @ inputs["W_hh"].T.astype(np.float64) + inputs["b_hh"]
    i_r, i_z, i_n = np.split(gi, 3, -1)
    h_r, h_z, h_n = np.split(gh, 3, -1)
    r = 1 / (1 + np.exp(-(i_r + h_r)))
    z = 1 / (1 + np.exp(-(i_z + h_z)))
    n = np.tanh(i_n + r * h_n)
    h_new = (1 - z) * n + z * hid
    outr = h_new @ inputs["Wfc"][0].astype(np.float64) + inputs["bfc"][0]
    err_h = np.abs(hnewT.T - h_new) / (np.abs(h_new) + 1e-3)
    err_o = np.abs(fc[:, 0] - outr) / (np.abs(outr) + 1e-3)
    print("h_new rel err:", err_h.max(), "fc rel err:", err_o.max())
